# revision 28
# baseline (speedup 1.0000x reference)
"""GQA kernel for Trainium2, 8 NeuronCores.

Problem: nn_GroupQueryAttention — B=4, S=2048, E=2048, 16 heads / 4 groups,
d_head=128.  out = softmax((x@Wq) (x@Wk)^T / sqrt(d)) (x@Wv) @ Wo + biases.

Sharding: core c -> (batch b = c//2, half = c%2).  Each core handles one
batch and 2 of the 4 KV groups (= 8 of the 16 heads): Wq columns / Wo rows
split by head, Wk/Wv columns split by group.  Each core produces a partial
output projection for its batch; the host sums the two halves (bo is folded
into half 0).

Layout: inputs are fed pre-transposed (x^T [E,S]) in bf16; projection
weights and the K/V input streams are host-packed partition-major so every
SBUF load is one DMA with multi-KB contiguous descriptors.  Intermediate
activations stay f32r:
  qh^T[d,s] = Wq_h^T x^T      kT[d,t] = Wk_g^T x^T      vT[d,t] = Wv_g^T x^T
  vh[t,d]   = PE-transpose of vT (so attn·V gets a [t,d] stationary)
  s^T[t,q]  = kT.T·qh^T       e^T = exp(s^T/sqrt(d))
  ctxu^T[d,q] = vh.T·e^T      rsum[*,q] = ones.T·(pairwise-summed e^T)
  ctx^T = ctxu^T * (1/rsum)   out[s,e] = ctx^T.T·Wo + bo
Softmax skips the max-subtraction: scores are ~N(0,1), far from fp32 exp
overflow.

Perf structure: phase C is a single flat software pipeline across every
(head, q-tile, t-pair) step — the attn·V + rowsum matmuls for step i are
emitted after the score matmuls of step i+1, so the PE stream never drains
waiting on the scalar engine's exp (and stays at full p-state clock).  Exp
tiles are pair-summed (DVE for 6/8 steps, GpSimd for 2/8 with their rowsum
matmuls deferred two steps to keep GpSimd's slow semaphore turnaround off
the critical path), halving the rowsum matmul rows.
"""

import sys

sys.path.insert(0, "/opt/trn_rl_repo")

import ml_dtypes
import numpy as np

B, S, E = 4, 2048, 2048
D = 128            # head dim
HPC = 8            # heads per core
GPC = 2            # groups per core
QC = HPC * D       # 1024 Wq cols per core
KV = GPC * D       # 256 Wk/Wv cols per core
NE = E // D        # 16 contraction chunks
ST = S // 512      # 4 tiles of 512 along s (q rows)
TT = S // 1024     # 2 tiles of 1024 along t (phase A streaming)
NT = S // D        # 16 t-chunks of 128
N_CORES = 8

GPS_TPS = (1, 3)   # t-pair steps whose exp pair-sum runs on GpSimd

_PROGRAM = None


def _build():
    from contextlib import ExitStack

    import concourse.bass as bass
    import concourse.mybir as mybir
    import concourse.tile as tile
    from concourse import bacc

    F32 = mybir.dt.float32
    F32R = mybir.dt.float32r
    BF16 = mybir.dt.bfloat16
    Exp = mybir.ActivationFunctionType.Exp
    SCALE = 1.0 / float(np.sqrt(D))

    nc = bacc.Bacc("TRN2", target_bir_lowering=False, debug=False)
    # x streams are packed tt-major: row tt*D+p holds chunks n=0..15 of
    # 1024 cols each, contiguous per partition row.
    xqp = nc.dram_tensor("xqp", [ST * D, NE * 512], BF16, kind="ExternalInput")
    xkp = nc.dram_tensor("xkp", [TT * D, NE * 1024], BF16, kind="ExternalInput")
    xvp = nc.dram_tensor("xvp", [TT * D, NE * 1024], BF16, kind="ExternalInput")
    wkp = nc.dram_tensor("wkp", [D, NE * KV], BF16, kind="ExternalInput")
    wvp = nc.dram_tensor("wvp", [D, NE * KV], BF16, kind="ExternalInput")
    wqp = nc.dram_tensor("wqp", [D, NE * QC], BF16, kind="ExternalInput")
    wop = nc.dram_tensor("wop", [D, HPC * E], F32R, kind="ExternalInput")
    cstb = nc.dram_tensor("cstb", [D, 12], F32, kind="ExternalInput")
    cstm = nc.dram_tensor("cstm", [D, 2 * D], F32R, kind="ExternalInput")
    bo = nc.dram_tensor("bo", [E], F32, kind="ExternalInput")
    out = nc.dram_tensor("out_p", [S, E], F32, kind="ExternalOutput")

    def bcast(dram, n):
        return bass.AP(tensor=dram.ap().tensor, offset=0, ap=[[0, D], [1, n]])

    with tile.TileContext(nc) as tc:
        with ExitStack() as top:
            const = top.enter_context(tc.tile_pool(name="const", bufs=1))
            acts = top.enter_context(tc.tile_pool(name="acts", bufs=1))

            cstb_sb = const.tile([D, 12], F32)
            nc.sync.dma_start(out=cstb_sb, in_=cstb.ap())
            cstm_sb = const.tile([D, 2 * D], F32R)
            nc.sync.dma_start(out=cstm_sb, in_=cstm.ap())
            bq_sb = cstb_sb[:, 0:HPC]
            bk_sb = cstb_sb[:, HPC : HPC + GPC]
            bv_sb = cstb_sb[:, HPC + GPC : HPC + 2 * GPC]
            ones_sb = cstm_sb[:, 0:D]
            id_sb = cstm_sb[:, D : 2 * D]

            # persistent activations: k^T per group, vh per t-subtile,
            # qh^T / ctx^T share one 9-slot group (qh[h] dies as cx[h] is born)
            kT = [acts.tile([D, S], F32R, name=f"kT{g}") for g in range(GPC)]
            vh = [acts.tile([D, KV], F32R, name=f"vh{t}") for t in range(NT)]

            def qcx_tile(name):
                return acts.tile([D, S], F32R, name=name, tag="qcx", bufs=9)

            # ---- Phases A (K/V proj) + B (Q proj) ----
            qh = []
            with ExitStack() as ab:
                pa = ab.enter_context(tc.tile_pool(name="pa", bufs=2))
                pb = ab.enter_context(tc.tile_pool(name="pb", bufs=2))

                # weights ride the scalar-engine queue (K/V first — needed by
                # the first matmul) so the sync queue's x-chunk stream starts
                # transferring immediately
                # wk/wv split in halves so the first matmul's weights land
                # a few us sooner
                wk_sb = pa.tile([D, NE, KV], BF16, bufs=1)
                wv_sb = pa.tile([D, NE, KV], BF16, bufs=1)
                kv_r = wkp.ap().rearrange("p (n c) -> p n c", n=NE)
                vv_r = wvp.ap().rearrange("p (n c) -> p n c", n=NE)
                nc.scalar.dma_start(out=wk_sb[:, : NE // 2], in_=kv_r[:, : NE // 2])
                nc.scalar.dma_start(out=wv_sb[:, : NE // 2], in_=vv_r[:, : NE // 2])
                nc.scalar.dma_start(out=wk_sb[:, NE // 2 :], in_=kv_r[:, NE // 2 :])
                nc.scalar.dma_start(out=wv_sb[:, NE // 2 :], in_=vv_r[:, NE // 2 :])
                wq_sb = pb.tile([D, NE, QC], BF16, bufs=1)
                nc.scalar.dma_start(
                    out=wq_sb, in_=wqp.ap().rearrange("p (n c) -> p n c", n=NE)
                )

                vT_sb = [
                    pa.tile([D, S], F32R, name=f"vT{g}", bufs=1) for g in range(GPC)
                ]

                with tc.tile_pool(name="psa", bufs=1, space="PSUM") as psa:
                    for tt in range(TT):
                        ps_k = [
                            psa.tile(
                                [D, 1024], F32, name=f"ps_k{g}_{tt}", tag=f"psk{g}"
                            )
                            for g in range(GPC)
                        ]
                        ps_vT = [
                            psa.tile(
                                [D, 1024], F32, name=f"ps_vT{g}_{tt}", tag=f"psv{g}"
                            )
                            for g in range(GPC)
                        ]
                        for e in range(NE):
                            xk_ch = pa.tile(
                                [D, 1024], BF16, name=f"xk_{tt}_{e}", tag="xk_ch",
                                bufs=4,
                            )
                            nc.sync.dma_start(
                                out=xk_ch,
                                in_=xkp.ap()[
                                    tt * D : (tt + 1) * D,
                                    e * 1024 : (e + 1) * 1024,
                                ],
                            )
                            xv_ch = pa.tile(
                                [D, 1024], BF16, name=f"xv_{tt}_{e}", tag="xv_ch",
                                bufs=4,
                            )
                            nc.sync.dma_start(
                                out=xv_ch,
                                in_=xvp.ap()[
                                    tt * D : (tt + 1) * D,
                                    e * 1024 : (e + 1) * 1024,
                                ],
                            )
                            # matmul out must fit one PSUM bank (512 f32), so
                            # each 1024-wide chunk is two matmuls sharing one
                            # LDWEIGHTS of the stationary
                            for g in range(GPC):
                                for hf in range(2):
                                    nc.tensor.matmul(
                                        ps_k[g][:, hf * 512 : (hf + 1) * 512],
                                        wk_sb[:, e, g * D : (g + 1) * D],
                                        xk_ch[:, hf * 512 : (hf + 1) * 512],
                                        start=(e == 0),
                                        stop=(e == NE - 1),
                                    )
                            for g in range(GPC):
                                for hf in range(2):
                                    nc.tensor.matmul(
                                        ps_vT[g][:, hf * 512 : (hf + 1) * 512],
                                        wv_sb[:, e, g * D : (g + 1) * D],
                                        xv_ch[:, hf * 512 : (hf + 1) * 512],
                                        start=(e == 0),
                                        stop=(e == NE - 1),
                                    )
                        for g in range(GPC):
                            nc.vector.tensor_scalar_add(
                                out=kT[g][:, tt * 1024 : (tt + 1) * 1024],
                                in0=ps_k[g],
                                scalar1=bk_sb[:, g : g + 1],
                            )
                            nc.vector.tensor_scalar_add(
                                out=vT_sb[g][:, tt * 1024 : (tt + 1) * 1024],
                                in0=ps_vT[g],
                                scalar1=bv_sb[:, g : g + 1],
                            )

                # transposes vT -> vh, bunched after psa closes (pstr banks
                # reuse psa's); the DVE copies overlap phase B's matmuls
                with tc.tile_pool(name="psb", bufs=2, space="PSUM") as psb:
                    for g in range(GPC):
                        for t_c in range(NT):
                            ps_tr = psb.tile(
                                [D, D], F32R, name=f"ps_tr{g}_{t_c}",
                                tag="pstr", bufs=2,
                            )
                            nc.tensor.transpose(
                                ps_tr, vT_sb[g][:, t_c * D : (t_c + 1) * D], id_sb
                            )
                            nc.vector.tensor_copy(
                                out=vh[t_c][:, g * D : (g + 1) * D], in_=ps_tr
                            )

                    # ---- Phase B: Q projection, wq resident, h-outer ----
                    for st in range(ST):
                        xq_ch = []
                        for e in range(NE):
                            t_ = pb.tile(
                                [D, 512], BF16, name=f"xq_{st}_{e}", tag="xq_ch",
                                bufs=22,
                            )
                            nc.scalar.dma_start(
                                out=t_,
                                in_=xqp.ap()[
                                    st * D : (st + 1) * D, e * 512 : (e + 1) * 512
                                ],
                            )
                            xq_ch.append(t_)
                        for h in range(HPC):
                            ps_q = psb.tile(
                                [D, 512], F32, name=f"ps_q{st}_{h}", tag="psq", bufs=2
                            )
                            for e in range(NE):
                                nc.tensor.matmul(
                                    ps_q,
                                    wq_sb[:, e, h * D : (h + 1) * D],
                                    xq_ch[e],
                                    start=(e == 0),
                                    stop=(e == NE - 1),
                                )
                            if st == 0:
                                qh.append(qcx_tile(f"qh{h}"))
                            nc.vector.tensor_scalar_add(
                                out=qh[h][:, st * 512 : (st + 1) * 512],
                                in0=ps_q,
                                scalar1=bq_sb[:, h : h + 1],
                            )

            # ---- Phases C (attention) + D (output projection) ----
            cx = []
            with ExitStack() as cd:
                pw = cd.enter_context(tc.tile_pool(name="pw", bufs=1))
                # prefetch D-phase weights during C (sync queue idle in C)
                wo_sb = pw.tile([D, HPC, E], F32R)
                nc.sync.dma_start(
                    out=wo_sb, in_=wop.ap().rearrange("p (c e) -> p c e", c=HPC)
                )
                bo_rep = pw.tile([D, E], F32)
                nc.scalar.dma_start(out=bo_rep, in_=bcast(bo, E))

                with ExitStack() as cc:
                    pc = cc.enter_context(tc.tile_pool(name="pc", bufs=2))
                    psc = cc.enter_context(
                        tc.tile_pool(name="psc", bufs=2, space="PSUM")
                    )

                    # One flat software pipeline over all (h, qt, tp) steps:
                    # attn·V + rowsum of step i issue after the scores of
                    # step i+1, crossing qt/head boundaries, so the PE never
                    # waits on exp.  `deferred_rs` holds GpSimd pair-sums'
                    # rowsum matmuls for two extra steps.
                    pending = None   # (h, qt, tp, ex, ps_ctx, ps_rs, g)
                    # the end-of-qt reciprocal+multiply drain is deferred into
                    # the NEXT qt's pipeline (emitted at its tp==2 flush) so
                    # it never delays the boundary, and the reciprocal is
                    # split in halves to keep DVE queue latency low
                    pending_drain = []  # [(h, qt, ps_ctx, ps_rs)]

                    def emit_drains():
                        while pending_drain:
                            d_h, d_qt, d_ctx, d_rs = pending_drain.pop(0)
                            rr = pc.tile(
                                [D, 512], F32, name=f"rr{d_h}_{d_qt}", tag="rr",
                                bufs=2,
                            )
                            nc.vector.reciprocal(out=rr[:, 0:256], in_=d_rs[:, 0:256])
                            nc.vector.reciprocal(
                                out=rr[:, 256:512], in_=d_rs[:, 256:512]
                            )
                            nc.vector.tensor_mul(
                                out=cx[d_h][:, d_qt * 512 : (d_qt + 1) * 512],
                                in0=d_ctx,
                                in1=rr,
                            )

                    def flush_pending():
                        nonlocal pending
                        if pending is None:
                            return
                        h, qt, tp, ex, ps_ctx, ps_rs, g = pending
                        last = NT // 2 - 1
                        for j in range(2):
                            t_c = tp * 2 + j
                            nc.tensor.matmul(
                                ps_rs,
                                ones_sb,
                                ex[:, j, :],
                                start=(t_c == 0),
                                stop=(t_c == NT - 1),
                            )
                        for j in range(2):
                            t_c = tp * 2 + j
                            nc.tensor.matmul(
                                ps_ctx,
                                vh[t_c][:, g * D : (g + 1) * D],
                                ex[:, j, :],
                                start=(t_c == 0),
                                stop=(t_c == NT - 1),
                            )
                        if tp == 2:
                            emit_drains()
                        if tp == last:
                            pending_drain.append((h, qt, ps_ctx, ps_rs))
                        pending = None

                    for h in range(HPC):
                        g = h // (HPC // GPC)
                        cx.append(qcx_tile(f"cx{h}"))
                        for qt in range(ST):
                            qsl = qh[h][:, qt * 512 : (qt + 1) * 512]
                            # allocate ps_sT first so its banks sit at the
                            # bottom of the psum region (phase D's ps_o then
                            # reuses banks whose last reader is exp, not the
                            # end-of-head reciprocal)
                            ps_sT0 = psc.tile(
                                [D, 2, 512], F32, name=f"ps_sT{h}_{qt}_0", tag="ps_sT"
                            )
                            ps_ctx = psc.tile(
                                [D, 512], F32, name=f"ps_ctx{h}_{qt}", tag="ps_ctx"
                            )
                            ps_rs = psc.tile(
                                [D, 512], F32, name=f"ps_rs{h}_{qt}", tag="ps_rs"
                            )
                            for tp in range(NT // 2):
                                ps_sT = ps_sT0 if tp == 0 else psc.tile(
                                    [D, 2, 512], F32, name=f"ps_sT{h}_{qt}_{tp}",
                                    tag="ps_sT",
                                )
                                for j in range(2):
                                    t_c = tp * 2 + j
                                    nc.tensor.matmul(
                                        ps_sT[:, j, :],
                                        kT[g][:, t_c * D : (t_c + 1) * D],
                                        qsl,
                                        start=True,
                                        stop=True,
                                    )
                                flush_pending()
                                ex = pc.tile(
                                    [D, 2, 512], F32R, name=f"ex{h}_{qt}_{tp}",
                                    tag="ex", bufs=3,
                                )
                                nc.scalar.activation(
                                    out=ex, in_=ps_sT, func=Exp, scale=SCALE
                                )
                                pending = (h, qt, tp, ex, ps_ctx, ps_rs, g)
                            # no flush here: `pending` carries into the next
                            # qt's first flush so the boundary never waits on
                            # the scalar engine's exp backlog
                    flush_pending()
                    emit_drains()

                # ---- Phase D: output projection + bias ----
                with ExitStack() as dd:
                    pd = dd.enter_context(tc.tile_pool(name="pd", bufs=2))
                    psd = dd.enter_context(
                        tc.tile_pool(name="psd", bufs=3, space="PSUM")
                    )
                    for et in range(ST):
                        for ss in range(NT):
                            ps_o = psd.tile(
                                [D, 512], F32, name=f"ps_o{et}_{ss}", tag="ps_o"
                            )
                            for hh in range(HPC):
                                nc.tensor.matmul(
                                    ps_o,
                                    cx[hh][:, ss * D : (ss + 1) * D],
                                    wo_sb[:, hh, et * 512 : (et + 1) * 512],
                                    start=(hh == 0),
                                    stop=(hh == HPC - 1),
                                )
                            ot = pd.tile(
                                [D, 512], F32, name=f"ot{et}_{ss}", tag="ot", bufs=3
                            )
                            nc.vector.tensor_add(
                                out=ot,
                                in0=ps_o,
                                in1=bo_rep[:, et * 512 : (et + 1) * 512],
                            )
                            nc.sync.dma_start(
                                out=out.ap()[
                                    ss * D : (ss + 1) * D, et * 512 : (et + 1) * 512
                                ],
                                in_=ot,
                            )

    nc.compile()
    return nc


def _get_program():
    global _PROGRAM
    if _PROGRAM is None:
        _PROGRAM = _build()
    return _PROGRAM


def make_in_maps(q, k, v, Wq, bq, Wk, bk, Wv, bv, Wo, bo):
    f32 = lambda a: np.asarray(a, dtype=np.float32)
    bf16 = lambda a: np.ascontiguousarray(a).astype(ml_dtypes.bfloat16)
    q, k, v = f32(q), f32(k), f32(v)
    Wq, bq, Wk, bk, Wv, bv, Wo, bo = (
        f32(Wq), f32(bq), f32(Wk), f32(bk), f32(Wv), f32(bv), f32(Wo), f32(bo)
    )

    def pack_x(xt, width):
        # x^T [E,S] -> [S//width * D, NE * width], row tt*D+p contiguous
        n_t = S // width
        return (
            xt.reshape(NE, D, n_t, width)
            .transpose(2, 1, 0, 3)
            .reshape(n_t * D, NE * width)
        )

    in_maps = []
    for c in range(N_CORES):
        b, half = c // 2, c % 2
        wq_h = Wq[:, half * QC : (half + 1) * QC]      # [E, QC]
        wk_h = Wk[:, half * KV : (half + 1) * KV]      # [E, KV]
        wv_h = Wv[:, half * KV : (half + 1) * KV]
        wo_h = Wo[half * QC : (half + 1) * QC, :]      # [QC, E]
        wk_p = wk_h.reshape(NE, D, KV).transpose(1, 0, 2).reshape(D, NE * KV)
        wv_p = wv_h.reshape(NE, D, KV).transpose(1, 0, 2).reshape(D, NE * KV)
        wq_p = wq_h.reshape(NE, D, QC).transpose(1, 0, 2).reshape(D, NE * QC)
        wo_p = wo_h.reshape(HPC, D, E).transpose(1, 0, 2).reshape(D, HPC * E)
        bq_h = bq[half * QC : (half + 1) * QC].reshape(HPC, D).T      # [D, HPC]
        bk_h = bk[half * KV : (half + 1) * KV].reshape(GPC, D).T
        bv_h = bv[half * KV : (half + 1) * KV].reshape(GPC, D).T
        cstb = np.concatenate([bq_h, bk_h, bv_h], axis=1)
        cstm = np.concatenate(
            [np.ones((D, D), np.float32), np.eye(D, dtype=np.float32)], axis=1
        )
        in_maps.append(
            {
                "xqp": bf16(pack_x(q[b].T, 512)),
                "xkp": bf16(pack_x(k[b].T, 1024)),
                "xvp": bf16(pack_x(v[b].T, 1024)),
                "wkp": bf16(wk_p),
                "wvp": bf16(wv_p),
                "wqp": bf16(wq_p),
                "wop": np.ascontiguousarray(wo_p),
                "cstb": np.ascontiguousarray(cstb),
                "cstm": np.ascontiguousarray(cstm),
                "bo": bo if half == 0 else np.zeros_like(bo),
            }
        )
    return in_maps


def combine_results(results):
    out = np.empty((B, S, E), np.float32)
    for b in range(B):
        out[b] = np.asarray(results[2 * b]["out_p"]) + np.asarray(
            results[2 * b + 1]["out_p"]
        )
    return out


def kernel(q, k, v, Wq, bq, Wk, bk, Wv, bv, Wo, bo):
    from concourse.bass_utils import run_bass_kernel_spmd

    nc = _get_program()
    in_maps = make_in_maps(q, k, v, Wq, bq, Wk, bk, Wv, bv, Wo, bo)
    res = run_bass_kernel_spmd(nc, in_maps, core_ids=list(range(N_CORES)))
    return combine_results(res.results)
